# revision 32
# baseline (speedup 1.0000x reference)
"""CrossModalFusion Trainium2 kernel (fp8 DoubleRow + folded-QK edition).

Reference computation (per batch b):
    q = rgb @ Wq + bq                 [S, H]
    k = pose @ Wk + bk                [S, H]
    v = pose @ Wv + bv                [S, H]
    attn = softmax(q @ k.T / sqrt(H)) [S, S]
    out  = attn @ v                   [S, H]
    proj = out @ Wp + bp              [S, D]
    x = rgb + gate * proj
    fused = LayerNorm(x) * gamma + beta

Sharding: pure data-parallel over batch B=32 across 8 NeuronCores
(4 batches per core), identical SPMD program, no collectives.

Key algebra: q k^T = (rgb M) pose^T + r 1^T + 1 ck^T + bq.bk, with
M = Wq Wk^T [D, D], r = rgb (Wq bk), ck = pose (Wk bq).  The per-query
terms (r, bq.bk) scale whole softmax rows and cancel exactly against
the row normalization, so they are dropped.  The per-key term ck rides
the exp's per-partition bias.  This removes the entire k projection:
scores contract over D=400 (via poseT) instead of H=512 (via kT), and
kT is never materialized.

All large matmuls run in fp8e4 (e4m3) with MatmulPerfMode.DoubleRow
(two K-tiles per pass, out = A.T@X0 + B.T@X1, 2x the bf16/f32r row
rate).  e4m3 overflows to inf above 240, so ranges are managed:
  - M/Wv/Wp are prescaled by 16 into fp8 normal range; the factor is
    removed via scale=1/16 on the PSUM->SBUF copies (GT, v) and via
    the per-row normalization scale (proj).
  - attnT = exp(scores/sqrt(H) - ln 8): keeps exp under 240 even for
    6.3-sigma scores; cancels in the softmax normalization.
  - v additionally carries 1/4 (VSCALE) so outT = attnT@v stays under
    240 for rows with concentrated attention; undone in gr.

Per-core dataflow (per batch):
  - cast pose tiles to bf16, PE-transpose -> poseT fp8 [d, S] (d on
    partitions, 4 chunks of K=100 so d=400 needs no zero padding).
  - ckv[S] = z^T poseT duals (z = Wk bq), scattered to partitions and
    merged with the exp bias.
  - v[S, h] seq-major (lhsT for attn@v), bias via DVE, fp8 out.
  - per 512-column query block:
      rgbT via PE transposes; GT = (rgb M)^T via M-duals (ACT copy
        with 1/16, fp8);
      scoresT[sk, sq] = poseT-duals.T @ GT (2 duals over D);
      exp on ACT, scale + per-key bias fused, fp8 out, unnormalized;
      column sums: gpsimd accumulates attnT tiles (f32), one bf16
        ones-matmul reduces partitions -> csum;
      outT[h, sq] = v-duals x attnT (8 duals over S);
      proj[sq, d] = outT-duals x Wp (2 duals over H), proj emitted
        before the rank-1 csum scatters so the PE never waits on the
        DVE/gpsimd csum chain; softmax normalization, gate and the
        fp8 prescales fold into the per-row scale gr;
      fused residual + LayerNorm (bn_stats/bn_aggr, gamma/beta on
        gpsimd) and store.
"""

import numpy as np

B, S, D, H = 32, 2048, 400, 512
N_CORES = 8
B_LOC = B // N_CORES
LN_EPS = 1e-5
P = 128          # partitions
QBLK = 512       # query block (columns of scoresT)
NBLK = 512       # free-dim block for feature-major matmuls
DK = 100         # d-chunk partition size (4*100 = 400, no padding)
WSCALE = 16.0    # fp8 prescale on weights
VSCALE = 4.0     # extra downscale on v (keeps outT under fp8 overflow)
EXPB = -np.log(8.0)  # exp output scale (cancels in normalization)

WEIGHT_NAMES = ("Wq", "bq", "Wk", "bk", "Wv", "bv", "Wp", "bp",
                "ln_gamma", "ln_beta", "gate")


def build_nc(b_loc=B_LOC, s=S, d=D, h=H):
    import concourse.bass as bass
    import concourse.mybir as mybir
    import concourse.tile as tile
    from concourse import bacc
    from concourse.masks import make_identity

    def bcast(ap1d, p=P):
        """Broadcast a 1-D DRAM AP across p partitions (step-0 leading dim)."""
        return bass.AP(tensor=ap1d.tensor, offset=ap1d.offset,
                       ap=[[0, p]] + list(ap1d.ap))

    f32 = mybir.dt.float32
    bf16 = mybir.dt.bfloat16
    fp8 = mybir.dt.float8e4
    AF = mybir.ActivationFunctionType
    DR = mybir.MatmulPerfMode.DoubleRow

    nt = s // P              # seq tiles
    nqb = s // QBLK          # query blocks
    tpb = QBLK // P          # seq tiles per query block
    nht = h // P             # h tiles (partition chunks of H)
    nhd = nht // 2           # h duals
    ndc = d // DK            # d chunks (K=100)
    ndd = ndc // 2           # d duals
    ncd = nt // 2            # seq duals (attn@v contraction)
    scale = 1.0 / float(np.sqrt(h))

    nc = bacc.Bacc("TRN2", target_bir_lowering=False, debug=False,
                   num_swdge_queues=4)

    rgb = nc.dram_tensor("rgb", [b_loc, s, d], f32, kind="ExternalInput").ap()
    pose = nc.dram_tensor("pose", [b_loc, s, d], f32, kind="ExternalInput").ap()
    Wq = nc.dram_tensor("Wq", [d, h], f32, kind="ExternalInput").ap()
    bq = nc.dram_tensor("bq", [h], f32, kind="ExternalInput").ap()
    Wk = nc.dram_tensor("Wk", [d, h], f32, kind="ExternalInput").ap()
    bk = nc.dram_tensor("bk", [h], f32, kind="ExternalInput").ap()
    Wv = nc.dram_tensor("Wv", [d, h], f32, kind="ExternalInput").ap()
    bv = nc.dram_tensor("bv", [h], f32, kind="ExternalInput").ap()
    Wp = nc.dram_tensor("Wp", [h, d], f32, kind="ExternalInput").ap()
    bp = nc.dram_tensor("bp", [d], f32, kind="ExternalInput").ap()
    gamma = nc.dram_tensor("ln_gamma", [d], f32, kind="ExternalInput").ap()
    beta = nc.dram_tensor("ln_beta", [d], f32, kind="ExternalInput").ap()
    gate = nc.dram_tensor("gate", [1], f32, kind="ExternalInput").ap()
    out = nc.dram_tensor("out", [b_loc, s, d], f32, kind="ExternalOutput").ap()

    from contextlib import ExitStack

    with tile.TileContext(nc) as tc:
        with ExitStack() as ctx:
            pool = lambda **kw: ctx.enter_context(tc.tile_pool(**kw))
            const = pool(name="const", bufs=1)
            wpool = pool(name="wpool", bufs=1)
            praw = pool(name="praw", bufs=9)
            p16pool = pool(name="p16pool", bufs=3)    # bf16 casts of raw tiles
            ptp = pool(name="ptp", bufs=2)            # poseT (fp8)
            vtp = pool(name="vtp", bufs=2)            # v (fp8)
            rraw = pool(name="rraw", bufs=2 * tpb)
            rtp = pool(name="rtp", bufs=1)            # rgbT block (fp8)
            gtp = pool(name="gtp", bufs=1)            # GT block (fp8)
            atp = pool(name="atp", bufs=1)            # attnT (fp8)
            otp = pool(name="otp", bufs=1)            # outT block (fp8)
            wstage = pool(name="wstage", bufs=2)
            small = pool(name="small", bufs=4)
            cspool = pool(name="cspool", bufs=2)
            ypool = pool(name="ypool", bufs=2)
            ps_sc = pool(name="ps_sc", bufs=2, space="PSUM")
            ps_mm = pool(name="ps_mm", bufs=3, space="PSUM")
            ps_tr = pool(name="ps_tr", bufs=2, space="PSUM")
            ps_cs = pool(name="ps_cs", bufs=1, space="PSUM")
            # ---- constants (once per core) ----
            ident16 = const.tile([P, P], bf16)
            make_identity(nc, ident16)
            ones8 = const.tile([P, 2, 32], fp8)
            nc.vector.memset(ones8, 1.0)
            ones_11 = const.tile([1, 1], f32)
            nc.vector.memset(ones_11, 1.0)
            eps_sb = const.tile([P, 1], f32)
            nc.vector.memset(eps_sb, LN_EPS)
            expb_sb = const.tile([P, 1], f32)
            nc.vector.memset(expb_sb, EXPB)

            warm = ps_tr.tile([P, P], f32, tag="tr")
            for _ in range(200):
                nc.tensor.matmul(warm, ident16, ident16, start=True, stop=True)

            copy_i = 0  # alternate PSUM->SBUF copies between DVE and ACT

            def psum_copy(dst, src):
                nonlocal copy_i
                copy_i += 1
                if copy_i % 3 == 0:
                    nc.scalar.copy(out=dst, in_=src)
                else:
                    nc.vector.tensor_copy(out=dst, in_=src)

            def transpose_in(dst_tp, raw, dst_col0):
                """Cast raw [128, d] to bf16, PE-transpose, cast to fp8 in
                the PSUM->SBUF copy into dst_tp[:DK, c, dst_col0:+128]."""
                r16 = p16pool.tile([P, d], bf16, tag="r16")
                nc.vector.tensor_copy(out=r16, in_=raw)
                for c in range(ndc):
                    ps = ps_tr.tile([P, P], bf16, tag="tr")
                    nc.tensor.transpose(
                        ps[:DK, :], r16[:, c * DK:(c + 1) * DK], ident16)
                    psum_copy(dst_tp[:DK, c, dst_col0:dst_col0 + P],
                              ps[:DK, :])

            def emit_pose_dma(b, t0, t1):
                tiles = []
                for t in range(t0, t1):
                    po = praw.tile([P, d], f32, tag="praw")
                    nc.sync.dma_start(out=po, in_=pose[b, t * P:(t + 1) * P, :])
                    tiles.append(po)
                return tiles

            def emit_pose_tr(poseT, tiles, t0):
                if poseT is None:
                    poseT = ptp.tile([P, ndc, s], fp8, tag="poseT")
                for k, po in enumerate(tiles):
                    transpose_in(poseT, po, (t0 + k) * P)
                return poseT

            def emit_poseT(b, t0=0, t1=None, poseT=None):
                """pose[b] tiles [t0, t1) -> poseT (DMA + PE transposes)."""
                t1 = nt if t1 is None else t1
                return emit_pose_tr(poseT, emit_pose_dma(b, t0, t1), t0)

            def emit_gt(b, qb):
                """rgb block -> rgbT -> GT = (rgb M)^T; returns (GT, raws).

                Emitted between a block's scores and its attn@v so the PE
                has dense work while ACT runs the exp chain."""
                q0 = qb * QBLK
                rgbT = rtp.tile([P, ndc, QBLK], bf16, tag="rgbT")
                rgb_raw = []
                for j in range(tpb):
                    rr = rraw.tile([P, d], f32, tag="rraw")
                    nc.sync.dma_start(
                        out=rr, in_=rgb[b, q0 + j * P:q0 + (j + 1) * P, :])
                    transpose_in(rgbT, rr, j * P)
                    rgb_raw.append(rr)
                gT = gtp.tile([P, ndc, QBLK], fp8, tag="gT")
                for dc in range(ndc):
                    ps = ps_mm.tile([P, QBLK], f32, tag="mm")
                    for c in range(ndc):
                        nc.tensor.matmul(
                            ps[:DK, :],
                            m16[:DK, c, dc * DK:(dc + 1) * DK],
                            rgbT[:DK, c, :],
                            start=(c == 0), stop=(c == ndc - 1),
                        )
                    nc.scalar.copy(out=gT[:DK, dc, :], in_=ps[:DK, :])
                # residual base: rgb += gate*bp (after transposes read rgb)
                for j in range(tpb):
                    nc.vector.tensor_add(
                        out=rgb_raw[j], in0=rgb_raw[j], in1=bpg_bc)
                return gT, rgb_raw

            poseT = emit_poseT(0)  # pose DMAs launch before the weight loads

            # ---- weights (once per core) ----
            # Wq/Wk staged, cast bf16, PE-transposed -> WqT/WkT [h, d];
            # M = Wq Wk^T assembled with d-rows on partitions, fp8 x16.
            wqT = wstage.tile([P, nht, d], bf16, tag="wT")
            wkT = wstage.tile([P, nht, d], bf16, tag="wT")
            for dstT, W in ((wqT, Wq), (wkT, Wk)):
                wst = wstage.tile([P, ndc, h], f32, tag="wst")
                w16 = wstage.tile([P, ndc, h], bf16, tag="w16")
                for c in range(ndc):
                    nc.gpsimd.dma_start(
                        out=wst[:DK, c, :], in_=W[c * DK:(c + 1) * DK, :])
                nc.vector.tensor_copy(out=w16[:DK], in_=wst[:DK])
                for c in range(ndc):
                    for hc in range(nht):
                        ps = ps_tr.tile([P, P], bf16, tag="tr")
                        nc.tensor.transpose(
                            ps[:, :DK],
                            w16[:DK, c, hc * P:(hc + 1) * P],
                            ident16[:DK, :DK])
                        psum_copy(dstT[:, hc, c * DK:(c + 1) * DK],
                                  ps[:, :DK])
            m16 = wpool.tile([P, ndc, d], bf16)
            for dc in range(ndc):
                ps = ps_mm.tile([P, d], f32, tag="mm")
                for hc in range(nht):
                    nc.tensor.matmul(
                        ps[:DK, :],
                        wqT[:, hc, dc * DK:(dc + 1) * DK],
                        wkT[:, hc, :],
                        start=(hc == 0), stop=(hc == nht - 1),
                    )
                nc.vector.tensor_copy(out=m16[:DK, dc, :], in_=ps[:DK, :])

            # z = Wk @ bq (per-key score bias direction), fp8, padded free
            bq_sb = wpool.tile([P, nht], f32)
            nc.gpsimd.dma_start(out=bq_sb, in_=bq.rearrange("(t p) -> p t", p=P))
            bq16 = wpool.tile([P, nht], bf16)
            nc.vector.tensor_copy(out=bq16, in_=bq_sb)
            z8 = wpool.tile([P, ndc, 32], fp8)
            nc.gpsimd.memset(z8, 0.0)
            for dc in range(ndc):
                ps = ps_tr.tile([P, 1], f32, tag="tr")
                for hc in range(nht):
                    nc.tensor.matmul(
                        ps[:DK, :],
                        wkT[:, hc, dc * DK:(dc + 1) * DK],
                        bq16[:, hc:hc + 1],
                        start=(hc == 0), stop=(hc == nht - 1),
                    )
                nc.vector.tensor_copy(out=z8[:DK, dc, 0:1], in_=ps[:DK, :])

            wv8 = wpool.tile([P, ndc, h], fp8)
            wst = wstage.tile([P, ndc, h], f32, tag="wst")
            for c in range(ndc):
                nc.gpsimd.dma_start(
                    out=wst[:DK, c, :], in_=Wv[c * DK:(c + 1) * DK, :])
                nc.vector.tensor_scalar(
                    out=wv8[:DK, c, :], in0=wst[:DK, c, :],
                    scalar1=WSCALE, scalar2=None,
                    op0=mybir.AluOpType.mult,
                )
            wp8 = wpool.tile([P, nht, d], fp8)
            wstp = wstage.tile([P, nht, d], f32, tag="wstp")
            for t in range(nht):
                nc.gpsimd.dma_start(
                    out=wstp[:, t, :], in_=Wp[t * P:(t + 1) * P, :])
            nc.vector.tensor_scalar(
                out=wp8, in0=wstp, scalar1=WSCALE, scalar2=None,
                op0=mybir.AluOpType.mult,
            )

            # free-dim broadcasts
            bv_bc = wpool.tile([P, h], f32)
            nc.gpsimd.dma_start(out=bv_bc, in_=bcast(bv))
            bv4_bc = wpool.tile([P, h], f32)
            nc.vector.tensor_scalar(
                out=bv4_bc, in0=bv_bc, scalar1=1.0 / VSCALE, scalar2=None,
                op0=mybir.AluOpType.mult,
            )
            bp_bc = wpool.tile([P, d], f32)
            nc.gpsimd.dma_start(out=bp_bc, in_=bcast(bp))
            gamma_bc = wpool.tile([P, d], f32)
            nc.gpsimd.dma_start(out=gamma_bc, in_=bcast(gamma))
            beta_bc = wpool.tile([P, d], f32)
            nc.gpsimd.dma_start(out=beta_bc, in_=bcast(beta))
            gate_sb = wpool.tile([P, 1], f32)
            nc.gpsimd.dma_start(out=gate_sb, in_=bcast(gate))
            # bpg = gate * bp (added to rgb once per row tile)
            bpg_bc = wpool.tile([P, d], f32)
            nc.vector.tensor_scalar_mul(out=bpg_bc, in0=bp_bc, scalar1=gate_sb)
            # gr carries gate * VSCALE/WSCALE (leftover fp8 prescales)
            gate16 = wpool.tile([P, 1], f32)
            nc.vector.tensor_scalar(
                out=gate16, in0=gate_sb, scalar1=VSCALE / WSCALE, scalar2=None,
                op0=mybir.AluOpType.mult,
            )

            for b in range(b_loc):
                # ===== phase A: per-key exp bias ckb, v =====
                # ckv[S] = z^T poseT (per-key bias from bq), chunked
                ckb = small.tile([P, nt], f32, tag="ckb")
                for nb in range(s // NBLK):
                    cs = ps_cs.tile([1, NBLK], f32, tag="cs")
                    for dd in range(ndd):
                        nc.tensor.matmul(
                            cs,
                            z8[:DK, 2 * dd:2 * dd + 2, 0:1],
                            poseT[:DK, 2 * dd:2 * dd + 2,
                                  nb * NBLK:(nb + 1) * NBLK],
                            start=(dd == 0), stop=(dd == ndd - 1),
                            perf_mode=DR,
                        )
                    cksb = cspool.tile([1, NBLK], f32, tag="cksb")
                    nc.scalar.copy(out=cksb, in_=cs)
                    for jt in range(NBLK // P):
                        t = nb * (NBLK // P) + jt
                        pst = ps_tr.tile([P, 1], f32, tag="tr")
                        nc.tensor.matmul(
                            pst, cksb[0:1, jt * P:(jt + 1) * P], ones_11,
                            start=True, stop=True,
                        )
                        # ckb = scale*ckv + EXPB (exp bias, per partition)
                        nc.vector.tensor_scalar(
                            out=ckb[:, t:t + 1], in0=pst,
                            scalar1=scale, scalar2=EXPB,
                            op0=mybir.AluOpType.mult,
                            op1=mybir.AluOpType.add,
                        )

                v_sb = vtp.tile([P, nt, h], fp8, tag="v")
                for t in range(nt):
                    ps = ps_mm.tile([P, h], f32, tag="mm")
                    for dd in range(ndd):
                        nc.tensor.matmul(
                            ps,
                            poseT[:DK, 2 * dd:2 * dd + 2, t * P:(t + 1) * P],
                            wv8[:DK, 2 * dd:2 * dd + 2, :],
                            start=(dd == 0), stop=(dd == ndd - 1),
                            perf_mode=DR,
                        )
                    # v = (psum/WSCALE + bv)/VSCALE (free-dim bias), fp8 out
                    nc.vector.scalar_tensor_tensor(
                        out=v_sb[:, t, :], in0=ps,
                        scalar=1.0 / (WSCALE * VSCALE), in1=bv4_bc,
                        op0=mybir.AluOpType.mult, op1=mybir.AluOpType.add,
                    )

                # ============ phase B: query blocks (pipelined) ============
                gstate = emit_gt(b, 0)
                for qb in range(nqb):
                    q0 = qb * QBLK
                    gT, rgb_raw = gstate

                    # scoresT tiles [sk 128, sq QBLK]; exp -> attnT (fp8)
                    attnT = atp.tile([P, nt, QBLK], fp8, tag="attnT")
                    for c in range(nt):
                        ps = ps_sc.tile([P, QBLK], f32, tag="sc")
                        for dd in range(ndd):
                            nc.tensor.matmul(
                                ps,
                                poseT[:DK, 2 * dd:2 * dd + 2,
                                      c * P:(c + 1) * P],
                                gT[:DK, 2 * dd:2 * dd + 2, :],
                                start=(dd == 0), stop=(dd == ndd - 1),
                                perf_mode=DR,
                            )
                        nc.scalar.activation(
                            out=attnT[:, c, :], in_=ps, func=AF.Exp,
                            scale=scale, bias=ckb[:, c:c + 1])

                    # pipeline filler: PE builds the next GT (or the next
                    # batch's poseT) while ACT runs this block's exp chain.
                    if qb + 1 < nqb:
                        prefetch = (qb == nqb - 2 and b + 1 < b_loc)
                        if prefetch:
                            ptiles = emit_pose_dma(b + 1, 0, nt // 2)
                        gstate = emit_gt(b, qb + 1)
                        if prefetch:
                            next_poseT = emit_pose_tr(None, ptiles, 0)
                            ptiles2 = emit_pose_dma(b + 1, nt // 2, nt)
                    elif b + 1 < b_loc:
                        if nqb >= 2:
                            next_poseT = emit_pose_tr(
                                next_poseT, ptiles2, nt // 2)
                        else:
                            next_poseT = emit_poseT(b + 1)

                    # outT[h, sq] = sum_cd v-dual @ attnT-dual
                    outT = otp.tile([P, nht, QBLK], fp8, tag="outT")
                    for ht in range(nht):
                        ps = ps_mm.tile([P, QBLK], f32, tag="mm")
                        for cd in range(ncd):
                            nc.tensor.matmul(
                                ps,
                                v_sb[:, 2 * cd:2 * cd + 2,
                                     ht * P:(ht + 1) * P],
                                attnT[:, 2 * cd:2 * cd + 2, :],
                                start=(cd == 0), stop=(cd == ncd - 1),
                                perf_mode=DR,
                            )
                        psum_copy(outT[:, ht, :], ps)

                    # column sums of attnT: ones-duals, [1, QBLK]
                    cs = ps_cs.tile([1, QBLK], f32, tag="cs")
                    for cd in range(ncd):
                        nc.tensor.matmul(
                            cs, ones8[:, :, 0:1], attnT[:, 2 * cd:2 * cd + 2, :],
                            start=(cd == 0), stop=(cd == ncd - 1),
                            perf_mode=DR,
                        )
                    csum = cspool.tile([1, QBLK], f32, tag="csum")
                    nc.scalar.copy(out=csum, in_=cs)

                    # proj first (PE never waits on the csum chain), then
                    # the rank-1 scatter of csum for this j.
                    psps = []
                    for j in range(tpb):
                        psp = ps_mm.tile([P, d], f32, tag="mm")
                        for hd in range(nhd):
                            nc.tensor.matmul(
                                psp,
                                outT[:, 2 * hd:2 * hd + 2,
                                     j * P:(j + 1) * P],
                                wp8[:, 2 * hd:2 * hd + 2, :],
                                start=(hd == 0), stop=(hd == nhd - 1),
                                perf_mode=DR,
                            )
                        psps.append(psp)
                        pst = ps_tr.tile([P, 1], f32, tag="tr")
                        nc.tensor.matmul(
                            pst, csum[0:1, j * P:(j + 1) * P], ones_11,
                            start=True, stop=True,
                        )
                        rec = small.tile([P, 1], f32, tag="rec")
                        nc.vector.reciprocal(out=rec, in_=pst)
                        gr = small.tile([P, 1], f32, tag="gr")
                        nc.vector.tensor_mul(out=gr, in0=rec, in1=gate16)

                        # x = gr * proj + (rgb + gate*bp)
                        x = ypool.tile([P, d], f32, tag="x")
                        nc.vector.scalar_tensor_tensor(
                            out=x, in0=psp, scalar=gr, in1=rgb_raw[j],
                            op0=mybir.AluOpType.mult, op1=mybir.AluOpType.add,
                        )
                        # LayerNorm
                        stats = small.tile([P, 6], f32, tag="stats")
                        nc.vector.bn_stats(out=stats, in_=x)
                        mv = small.tile([P, 2], f32, tag="mv")
                        nc.vector.bn_aggr(out=mv, in_=stats)
                        sd = small.tile([P, 1], f32, tag="sd")
                        nc.scalar.activation(
                            out=sd, in_=mv[:, 1:2], func=AF.Sqrt, bias=eps_sb)
                        rstd = small.tile([P, 1], f32, tag="rstd")
                        nc.vector.reciprocal(out=rstd, in_=sd)
                        nc.vector.tensor_scalar(
                            out=x, in0=x, scalar1=mv[:, 0:1], scalar2=rstd,
                            op0=mybir.AluOpType.subtract,
                            op1=mybir.AluOpType.mult,
                        )
                        nc.vector.tensor_mul(out=x, in0=x, in1=gamma_bc)
                        nc.vector.tensor_add(out=x, in0=x, in1=beta_bc)
                        nc.sync.dma_start(
                            out=out[b, q0 + j * P:q0 + (j + 1) * P, :], in_=x)

                if b + 1 < b_loc:
                    poseT = next_poseT

    nc.compile()
    return nc


_CACHE = {}


def kernel(**inputs):
    from concourse.bass_utils import run_bass_kernel_spmd

    if "nc" not in _CACHE:
        _CACHE["nc"] = build_nc()
    nc = _CACHE["nc"]

    weights = {k: np.ascontiguousarray(inputs[k], dtype=np.float32)
               for k in WEIGHT_NAMES}
    rgb = np.ascontiguousarray(inputs["rgb"], dtype=np.float32)
    pose = np.ascontiguousarray(inputs["pose"], dtype=np.float32)

    in_maps = []
    for i in range(N_CORES):
        m = dict(weights)
        m["rgb"] = np.ascontiguousarray(rgb[i * B_LOC:(i + 1) * B_LOC])
        m["pose"] = np.ascontiguousarray(pose[i * B_LOC:(i + 1) * B_LOC])
        in_maps.append(m)

    res = run_bass_kernel_spmd(nc, in_maps, list(range(N_CORES))).results
    return np.concatenate([res[i]["out"] for i in range(N_CORES)], axis=0)
